# revision 10
# baseline (speedup 1.0000x reference)
"""Varlen causal attention (flash_attn_varlen semantics) on 8 Trainium2 cores.

Sharding: 16 heads across 8 cores (2 heads/core, Ulysses-style head shard,
identity comms). Each core runs the same SPMD Bass program on its head slice.

Per head: blocked attention over 128-row q blocks. For each q block only the
k blocks inside the (causal x segment) mask are computed -- the block structure
is specialized on the host from cu_seqlens at trace time. S = Q^T K runs in
float32r; P = exp(S * scale) in bf16 (logits are O(5), so no max subtraction
is needed); a ones-column appended to V yields the softmax denominator from
the same PV matmul.
"""

import numpy as np

L = 4096
H = 16
D = 128
N_CORES = 8
H_PER_CORE = H // N_CORES
SCALE = 1.0 / float(np.sqrt(D))
QB = 128  # q/k block size


def _seg_starts(cu: np.ndarray) -> np.ndarray:
    """Per-token segment start, exactly mirroring the reference searchsorted."""
    tok = np.arange(L)
    seg = np.searchsorted(cu[1:-1], tok, side="right")
    starts = np.concatenate([[0], cu[1:-1]])
    return starts[seg]


def _chunk_sizes(w: int) -> list:
    """Split w (multiple of 128) into matmul chunks <=512, avoiding <256
    trailing chunks (float32r runs at 1/4 rate below 256 free dim)."""
    sizes = [512] * (w // 512)
    rem = w % 512
    if rem:
        sizes.append(rem)
    if len(sizes) > 1 and sizes[-1] < 256:
        sizes[-2:] = [sizes[-2] - 128, sizes[-1] + 128]
    return sizes


def _build(cu: np.ndarray):
    import concourse.mybir as mybir
    import concourse.tile as tile
    from concourse import bacc
    from concourse.masks import make_identity

    f32 = mybir.dt.float32
    f32r = mybir.dt.float32r
    bf16 = mybir.dt.bfloat16
    AF = mybir.ActivationFunctionType

    seg_start = _seg_starts(cu)
    n_qb = L // QB

    nc = bacc.Bacc("TRN2", target_bir_lowering=False, debug=False,
                   num_devices=N_CORES)
    q_d = nc.dram_tensor("q", [L, H_PER_CORE, D], f32, kind="ExternalInput")
    k_d = nc.dram_tensor("k", [L, H_PER_CORE, D], f32, kind="ExternalInput")
    v_d = nc.dram_tensor("v", [L, H_PER_CORE, D], f32, kind="ExternalInput")
    o_d = nc.dram_tensor("out", [L, H_PER_CORE, D], f32, kind="ExternalOutput")

    with tile.TileContext(nc) as tc:
        with (
            tc.tile_pool(name="consts", bufs=1) as consts,
            tc.tile_pool(name="big", bufs=2) as big,
            tc.tile_pool(name="io", bufs=4) as io,
            tc.tile_pool(name="psb", bufs=3) as psb,
            tc.tile_pool(name="t_ps", bufs=2, space="PSUM") as tr_ps_pool,
            tc.tile_pool(name="s_ps", bufs=2, space="PSUM") as s_ps_pool,
            tc.tile_pool(name="o_ps", bufs=2, space="PSUM") as o_ps_pool,
        ):
            ident = consts.tile([128, 128], f32)
            make_identity(nc, ident[:])
            ident_bf = consts.tile([128, 128], bf16)
            nc.vector.tensor_copy(ident_bf[:], ident[:])

            for h in range(H_PER_CORE):
                # ---- prep: load + transpose Q,K; load + cast V (+ones col)
                qt_sb = big.tile([128, L], f32r, tag="qt")
                kt_sb = big.tile([128, L], f32r, tag="kt")
                v_sb = big.tile([128, n_qb, 132], bf16, tag="v")
                for t in range(n_qb):
                    r = slice(t * QB, (t + 1) * QB)
                    q_t = io.tile([128, D], f32, tag="q_in")
                    nc.gpsimd.dma_start(q_t[:], q_d[r, h, :])
                    tp = tr_ps_pool.tile([128, 128], f32, tag="t")
                    nc.tensor.transpose(tp[:], q_t[:], ident[:])
                    nc.vector.tensor_copy(qt_sb[:, r], tp[:])

                    k_t = io.tile([128, D], f32, tag="k_in")
                    nc.gpsimd.dma_start(k_t[:], k_d[r, h, :])
                    tp2 = tr_ps_pool.tile([128, 128], f32, tag="t")
                    nc.tensor.transpose(tp2[:], k_t[:], ident[:])
                    nc.vector.tensor_copy(kt_sb[:, r], tp2[:])

                    v_t = io.tile([128, D], f32, tag="v_in")
                    nc.gpsimd.dma_start(v_t[:], v_d[r, h, :])
                    nc.vector.memset(v_sb[:, t, 0:1], 1.0)
                    nc.vector.tensor_copy(v_sb[:, t, 1:129], v_t[:])

                # ---- main: per q block
                for i in range(n_qb):
                    q0 = i * QB
                    k_lo_b = int(seg_start[q0]) // QB
                    k_lo = k_lo_b * QB
                    w = (i + 1) * QB - k_lo

                    p_sb = psb.tile([128, L], bf16, tag="p")

                    # S = (Q^T)^T K^T, chunked; P = exp(S * scale)
                    c0 = 0
                    for cw in _chunk_sizes(w):
                        s_ps = s_ps_pool.tile([128, 512], f32)
                        nc.tensor.matmul(
                            s_ps[:, :cw],
                            qt_sb[:, q0:q0 + QB],
                            kt_sb[:, k_lo + c0:k_lo + c0 + cw],
                            start=True, stop=True,
                        )
                        nc.scalar.activation(p_sb[:, c0:c0 + cw], s_ps[:, :cw],
                                             AF.Exp, scale=SCALE)
                        c0 += cw

                    # segment-boundary masking: rows whose segment starts at
                    # b > k_lo must drop columns [k_lo, b). Zeroing those
                    # columns for all rows >= b works because later segments
                    # need a superset zeroed. Partition offsets must be
                    # 32-aligned, so row-conditional zeroing goes through
                    # affine_select (predicate on the partition index).
                    for b in sorted(set(int(s) for s in seg_start[q0:q0 + QB])):
                        ncols = b - k_lo
                        if ncols <= 0:
                            continue
                        rb = b - q0
                        if rb <= 0:
                            nc.vector.memset(p_sb[:, 0:ncols], 0.0)
                        else:
                            # keep row p iff p < rb  <=>  (rb-1-p) >= 0
                            nc.gpsimd.affine_select(
                                out=p_sb[:, 0:ncols], in_=p_sb[:, 0:ncols],
                                compare_op=mybir.AluOpType.is_ge, fill=0.0,
                                base=rb - 1, pattern=[[0, ncols]],
                                channel_multiplier=-1,
                            )

                    # causal triangle on the diagonal block
                    nc.gpsimd.affine_select(
                        out=p_sb[:, w - QB:w], in_=p_sb[:, w - QB:w],
                        compare_op=mybir.AluOpType.is_ge, fill=0.0,
                        base=0, pattern=[[-1, QB]], channel_multiplier=1,
                    )

                    # O[:, 0] = denom, O[:, 1:129] = P @ V
                    o_ps = o_ps_pool.tile([128, 129], f32)
                    for j in range(k_lo_b, i + 1):
                        pt_ps = tr_ps_pool.tile([128, 128], bf16, tag="t")
                        nc.tensor.transpose(
                            pt_ps[:], p_sb[:, (j - k_lo_b) * QB:(j - k_lo_b + 1) * QB],
                            ident_bf[:])
                        pt_sb = io.tile([128, 128], bf16, tag="pt")
                        nc.vector.tensor_copy(pt_sb[:], pt_ps[:])
                        nc.tensor.matmul(o_ps[:], pt_sb[:], v_sb[:, j, 0:129],
                                         start=(j == k_lo_b), stop=(j == i))

                    recip = io.tile([128, 1], f32, tag="recip")
                    nc.vector.reciprocal(recip[:], o_ps[:, 0:1])
                    o_sb = io.tile([128, D], f32, tag="o_out")
                    nc.vector.tensor_scalar_mul(o_sb[:], o_ps[:, 1:129], recip[:])
                    nc.gpsimd.dma_start(o_d[i * QB:(i + 1) * QB, h, :], o_sb[:])

    nc.compile()
    return nc


def _run(query, key, value, cu_seqlens, trace=False, **spmd_kwargs):
    from concourse import bass_utils

    query = np.ascontiguousarray(np.asarray(query, dtype=np.float32))
    key = np.ascontiguousarray(np.asarray(key, dtype=np.float32))
    value = np.ascontiguousarray(np.asarray(value, dtype=np.float32))
    cu = np.asarray(cu_seqlens, dtype=np.int64)

    nc = _build(cu)
    in_maps = []
    for c in range(N_CORES):
        hs = slice(c * H_PER_CORE, (c + 1) * H_PER_CORE)
        in_maps.append({
            "q": np.ascontiguousarray(query[:, hs, :]),
            "k": np.ascontiguousarray(key[:, hs, :]),
            "v": np.ascontiguousarray(value[:, hs, :]),
        })
    res = bass_utils.run_bass_kernel_spmd(nc, in_maps, list(range(N_CORES)),
                                          trace=trace, **spmd_kwargs)
    out = np.empty((L, H, D), dtype=np.float32)
    for c in range(N_CORES):
        out[:, c * H_PER_CORE:(c + 1) * H_PER_CORE, :] = res.results[c]["out"]
    return out, res


def kernel(query, key, value, cu_seqlens):
    out, _ = _run(query, key, value, cu_seqlens)
    return out
